# revision 1
# baseline (speedup 1.0000x reference)
"""Modulated 1x1 conv (ModConv) on 8 Trainium2 NeuronCores.

out[b,o,h,w] = sum_c (style[b,c] * weight[o,c]) * x[b,c,h,w]

Strategy: pure data parallel over the batch — 2 samples per core. Per
sample the kernel modulates the (pre-transposed) weight with the style
vector on DVE (cheap: [512,128] elements), then runs a K=512 contraction
as 4 PSUM-accumulated matmuls per 512-wide output tile. Matmul operands
use float32r (TF32-like PE path: full-rate rows vs 4 cycles/row for
fp32, ~1e-4 rel err), so the problem is HBM-bound (~21 MB/core at
~360 GB/s). x streams in as 1 MB [128, 2048] chunks alternating between
the SP and ACT HWDGE rings; outputs leave via the gpsimd SWDGE ring so
they never stall the input streams.
"""

import numpy as np

import concourse.bass as bass
import concourse.mybir as mybir
from concourse.bass_utils import run_bass_kernel_spmd
from concourse.tile import TileContext

B, CIN, COUT, H, W = 16, 512, 128, 64, 64
HW = H * W
N_CORES = 8
BPC = B // N_CORES  # samples per core
P = 128
KT = CIN // P  # k-tiles per contraction
NTILE = 512  # one PSUM bank of fp32
NT = HW // NTILE
NHALF = 2  # x chunks per k-tile (n-direction)
NCHUNK = HW // NHALF
FP32 = mybir.dt.float32
F32R = mybir.dt.float32r

# This container's walrus (public-SDK build) accepts at most one sync
# wait command per instruction; Tile's sem assignment attaches one wait
# per depended-on proc. Hoist the excess onto dedicated wait
# instructions (the same InstEventSemaphore a bass `wait_ge` emits)
# immediately before the over-subscribed instruction on its own engine.
MAX_WAITS_PER_INST = 1


def _split_sync_waits(nc: bass.Bass, limit: int = MAX_WAITS_PER_INST) -> int:
    n_split = 0
    for f in nc.m.functions:
        for bb in f.blocks:
            out = []
            for ins in bb.instructions:
                si = getattr(ins, "sync_info", None)
                if si is not None and si.on_wait and len(si.on_wait) > limit:
                    waits = list(si.on_wait)
                    for w in waits[:-limit]:
                        n_split += 1
                        es = mybir.InstEventSemaphore(
                            name=f"{ins.name}-ws{n_split}",
                            opcode="EventSemaphore",
                            engine=ins.engine,
                            sync_info=mybir.SyncInfo(on_wait=[w], on_update=[]),
                        )
                        nc.register_instruction(es, overwrite=True)
                        out.append(es)
                    si.on_wait = waits[-limit:]
                out.append(ins)
            bb.instructions[:] = out
    return n_split


def build_kernel(
    reps: int = 1,
    bench_mode: bool = False,
    nhalf: int = NHALF,
    x_bufs: int | None = None,
    psum_bufs: int = 4,
    skip_out: bool = False,
    skip_compute: bool = False,
    out_every: int | None = None,
    x_three_queues: bool = False,
    o_bufs: int = 2,
    pack: str = "packall16",  # "none"|"pack2"|"packall"|"packall8"|"packall16"
    out_hwdge: bool = False,
) -> bass.Bass:
    """reps>1 replicates the whole per-sample pipeline in-program (same
    inputs, outputs rewritten) — used only by the bench to measure
    steady-state per-iteration time with per-call overhead cancelled.
    bench_mode writes the big output to internal DRAM and exposes only a
    4-byte token output, so per-call tunnel traffic is negligible."""
    nchunk = HW // nhalf
    if x_bufs is None:
        # Exactly one slot of slack beyond 2 samples in flight, so the
        # HWDGE rings never stall on a slot release (packall 8 vs 9
        # measured ~55 vs ~35 us/iter; packall8 16 vs 17 similar; one
        # MORE slot regresses again in both cases).
        x_bufs = {
            "none": 2 * KT * nhalf,
            "pack2": 8,
            "packall": 9,
            "packall8": 17,
            "packall16": 33,
        }[pack]
    if out_every is None:
        out_every = NT // nhalf
    nc = bass.Bass()
    x = nc.dram_tensor("x", [BPC, CIN, HW], F32R, kind="ExternalInput")
    styleT = nc.dram_tensor("styleT", [CIN, BPC], FP32, kind="ExternalInput")
    wT = nc.dram_tensor("wT", [CIN, COUT], FP32, kind="ExternalInput")
    if bench_mode:
        out = nc.dram_tensor("out_scratch", [BPC, COUT, HW], FP32)
        token = nc.dram_tensor("token", [1, 1], FP32, kind="ExternalOutput")
    else:
        out = nc.dram_tensor("out", [BPC, COUT, HW], FP32, kind="ExternalOutput")
        token = None

    # The two HWDGE rings (SP + ACT) stream x in parallel.
    x_dma_engines = [nc.sync, nc.scalar]
    if x_three_queues:
        x_dma_engines = [nc.sync, nc.scalar, nc.gpsimd]

    with TileContext(nc) as tc:
        with (
            tc.tile_pool(name="consts", bufs=1) as cpool,
            tc.tile_pool(name="xs", bufs=x_bufs) as xpool,
            tc.tile_pool(name="os", bufs=o_bufs) as opool,
            tc.tile_pool(name="ps", bufs=psum_bufs, space="PSUM") as pspool,
        ):
            wT_sb = cpool.tile([P, KT, COUT], FP32)
            nc.sync.dma_start(out=wT_sb[:], in_=wT[:].rearrange("(t p) o -> p t o", p=P))
            sT_sb = cpool.tile([P, KT, BPC], FP32)
            nc.scalar.dma_start(
                out=sT_sb[:], in_=styleT[:].rearrange("(t p) b -> p t b", p=P)
            )
            # Per-sample modulated (transposed) weights: mw[p, b, t, o].
            # Stored as float32r so the PE takes the fast fp32 path.
            mw_sb = cpool.tile([P, BPC, KT, COUT], F32R)
            for b in range(BPC):
                for t in range(KT):
                    nc.vector.tensor_scalar_mul(
                        mw_sb[:, b, t, :], wT_sb[:, t, :], sT_sb[:, t, b : b + 1]
                    )

            dma_i = 0
            for _rep in range(reps):
                for b in range(BPC):
                    ntile, nt = NTILE, NT
                    if pack in ("packall", "packall8", "packall16"):
                        # One DMA per HW-chunk carrying all 4 k-tiles:
                        # packall: [128, 4, 1024] = 2 MB, feeds 2 n-tiles;
                        # packall8: [128, 4, 512] = 1 MB, feeds 1 n-tile;
                        # packall16: [128, 4, 256] = 512 KB, one 256-wide
                        # n-tile each (fp32r stays full-rate at N>=256).
                        qn = {"packall": 4, "packall8": 8, "packall16": 16}[pack]
                        qw = HW // qn
                        if pack == "packall16":
                            ntile, nt = 256, HW // 256
                        xq = []
                        for q in range(qn):
                            xt = xpool.tile([P, KT, qw], F32R, tag="xt")
                            eng = x_dma_engines[dma_i % len(x_dma_engines)]
                            dma_i += 1
                            eng.dma_start(
                                out=xt[:],
                                in_=x[b, :, q * qw : (q + 1) * qw].rearrange(
                                    "(t p) n -> p t n", p=P
                                ),
                            )
                            xq.append(xt)

                        def rhs(n, t, _ntile=ntile, _nper=None):
                            q, j = divmod(n, max(nt // qn, 1))
                            return xq[q][:, t, j * _ntile : (j + 1) * _ntile]
                    elif pack == "pack2":
                        # One DMA per (k-pair, HW-half): [128, 2, 2048] = 2 MB.
                        xg = [[None, None], [None, None]]
                        for h in range(2):
                            for g in range(2):
                                xt = xpool.tile([P, 2, HW // 2], F32R, tag="xt")
                                eng = x_dma_engines[dma_i % len(x_dma_engines)]
                                dma_i += 1
                                eng.dma_start(
                                    out=xt[:],
                                    in_=x[
                                        b,
                                        g * 2 * P : (g + 1) * 2 * P,
                                        h * (HW // 2) : (h + 1) * (HW // 2),
                                    ].rearrange("(t p) n -> p t n", p=P),
                                )
                                xg[g][h] = xt

                        def rhs(n, t):
                            h, j = divmod(n, NT // 2)
                            return xg[t // 2][h][
                                :, t % 2, j * NTILE : (j + 1) * NTILE
                            ]
                    else:
                        # x chunks: xh[t][h] = one (k-tile, HW-chunk) = 1 MB
                        xh = [[None] * nhalf for _ in range(KT)]
                        for h in range(nhalf):
                            for t in range(KT):
                                xt = xpool.tile([P, nchunk], F32R, tag="xt")
                                eng = x_dma_engines[dma_i % len(x_dma_engines)]
                                dma_i += 1
                                eng.dma_start(
                                    out=xt[:],
                                    in_=x[
                                        b,
                                        t * P : (t + 1) * P,
                                        h * nchunk : (h + 1) * nchunk,
                                    ],
                                )
                                xh[t][h] = xt

                        def rhs(n, t):
                            h, j = divmod(n, NT // nhalf)
                            return xh[t][h][:, j * NTILE : (j + 1) * NTILE]

                    if skip_compute:
                        continue
                    oev = out_every * (nt // NT)  # keep out-chunk BYTE size fixed
                    ot = opool.tile([P, HW], FP32, tag="ot")
                    for n in range(nt):
                        ps = pspool.tile([P, ntile], FP32, tag="ps")
                        for t in range(KT):
                            nc.tensor.matmul(
                                ps[:],
                                mw_sb[:, b, t, :],
                                rhs(n, t),
                                start=(t == 0),
                                stop=(t == KT - 1),
                            )
                        nc.vector.tensor_copy(
                            out=ot[:, n * ntile : (n + 1) * ntile], in_=ps[:]
                        )
                        if not skip_out and (n + 1) % oev == 0:
                            lo = (n + 1 - oev) * ntile
                            hi = (n + 1) * ntile
                            if out_hwdge:
                                oeng = x_dma_engines[dma_i % len(x_dma_engines)]
                                dma_i += 1
                            else:
                                oeng = nc.gpsimd
                            oeng.dma_start(out=out[b, :, lo:hi], in_=ot[:, lo:hi])
            if token is not None:
                nc.gpsimd.dma_start(out=token[:], in_=mw_sb[:1, 0, 0, :1])

    _split_sync_waits(nc)
    return nc


_NC_CACHE: bass.Bass | None = None


def _get_nc() -> bass.Bass:
    global _NC_CACHE
    if _NC_CACHE is None:
        _NC_CACHE = build_kernel()
    return _NC_CACHE


def make_in_maps(x: np.ndarray, style: np.ndarray, weight: np.ndarray):
    x_flat = np.ascontiguousarray(np.asarray(x, dtype=np.float32)).reshape(B, CIN, HW)
    styleT = np.ascontiguousarray(np.asarray(style, dtype=np.float32).T)  # [CIN, B]
    wT = np.ascontiguousarray(np.asarray(weight, dtype=np.float32).T)  # [CIN, COUT]
    in_maps = []
    for c in range(N_CORES):
        sl = slice(c * BPC, (c + 1) * BPC)
        in_maps.append(
            {
                "x": x_flat[sl],
                "styleT": np.ascontiguousarray(styleT[:, sl]),
                "wT": wT,
            }
        )
    return in_maps


def gather_out(results) -> np.ndarray:
    out = np.empty((B, COUT, H, W), dtype=np.float32)
    for c in range(N_CORES):
        out[c * BPC : (c + 1) * BPC] = results[c]["out"].reshape(BPC, COUT, H, W)
    return out


def kernel(x: np.ndarray, style: np.ndarray, weight: np.ndarray) -> np.ndarray:
    nc = _get_nc()
    in_maps = make_in_maps(x, style, weight)
    res = run_bass_kernel_spmd(nc, in_maps, core_ids=list(range(N_CORES)))
    return gather_out(res.results)



# revision 2
# speedup vs baseline: 1.8695x; 1.8695x over previous
"""Modulated 1x1 conv (ModConv) on 8 Trainium2 NeuronCores.

out[b,o,h,w] = sum_c (style[b,c] * weight[o,c]) * x[b,c,h,w]

Strategy: data parallel over the batch — 2 samples per core. The problem
is HBM-bound (per-core traffic dominates; HBM-per-NC limit ~358 GB/s),
so both x and the output travel as bf16 (correctness gate is 2e-2;
bf16 GEMM error is ~4e-3), halving traffic vs fp32: 8 MB in + 2 MB out
per core -> ~28 us roofline vs ~59 us for fp32. PE time (bf16 full
rate, 2 samples x 4 k-tiles x 4096 cols = 32768 cycles @ 2.4 GHz
~ 13.7 us) hides under the DMA.

The host pre-permutes x to [b, q, p, t, n] (q = 8 HW-chunks of 512
cols, p = partition = channel % 128, t = k-tile = channel // 128) so
every x chunk is one fully-contiguous 512 KB DMA (4 KB per partition).
Chunks alternate between the SP and ACT HWDGE rings; outputs leave as
bf16 via the gpsimd SWDGE ring so they never stall the input streams.
Per chunk: 4 PSUM-accumulated bf16 matmuls (N=512), DVE copies the
fp32 PSUM tile out as bf16.
"""

import numpy as np
import ml_dtypes

import concourse.bass as bass
import concourse.mybir as mybir
from concourse.bass_utils import run_bass_kernel_spmd
from concourse.tile import TileContext

B, CIN, COUT, H, W = 16, 512, 128, 64, 64
HW = H * W
N_CORES = 8
BPC = B // N_CORES  # samples per core
P = 128
KT = CIN // P  # k-tiles per contraction
NQ = 8  # HW-chunks per sample
NC = HW // NQ  # columns per chunk (= one PSUM bank of fp32)
FP32 = mybir.dt.float32
BF16 = mybir.dt.bfloat16
NP_BF16 = ml_dtypes.bfloat16

# This container's walrus (public-SDK build) accepts at most one sync
# wait command per instruction; Tile's sem assignment attaches one wait
# per depended-on proc. Hoist the excess onto dedicated wait
# instructions (the same InstEventSemaphore a bass `wait_ge` emits)
# immediately before the over-subscribed instruction on its own engine.
MAX_WAITS_PER_INST = 1


def _split_sync_waits(nc: bass.Bass, limit: int = MAX_WAITS_PER_INST) -> int:
    n_split = 0
    for f in nc.m.functions:
        for bb in f.blocks:
            out = []
            for ins in bb.instructions:
                si = getattr(ins, "sync_info", None)
                if si is not None and si.on_wait and len(si.on_wait) > limit:
                    waits = list(si.on_wait)
                    for w in waits[:-limit]:
                        n_split += 1
                        es = mybir.InstEventSemaphore(
                            name=f"{ins.name}-ws{n_split}",
                            opcode="EventSemaphore",
                            engine=ins.engine,
                            sync_info=mybir.SyncInfo(on_wait=[w], on_update=[]),
                        )
                        nc.register_instruction(es, overwrite=True)
                        out.append(es)
                    si.on_wait = waits[-limit:]
                out.append(ins)
            bb.instructions[:] = out
    return n_split


def build_kernel(
    reps: int = 1,
    bench_mode: bool = False,
    x_bufs: int = 2 * NQ + 1,  # 2 samples in flight + 1 slot of slack
    psum_bufs: int = 4,
    o_bufs: int = 2,
    out_every: int = 4,  # chunks per output DMA (4 -> 512 KB bf16)
) -> bass.Bass:
    """reps>1 replicates the whole per-sample pipeline in-program (same
    inputs, outputs rewritten) — used only by the bench to measure
    steady-state per-iteration time with per-call overhead cancelled.
    bench_mode writes the big output to internal DRAM and exposes only a
    4-byte token output, so per-call tunnel traffic is negligible."""
    nc = bass.Bass()
    # x chunk ci = b*NQ + q: [P, KT*NC] fully contiguous in DRAM.
    x = nc.dram_tensor("x", [BPC * NQ, P, KT * NC], BF16, kind="ExternalInput")
    styleT = nc.dram_tensor("styleT", [CIN, BPC], FP32, kind="ExternalInput")
    wT = nc.dram_tensor("wT", [CIN, COUT], FP32, kind="ExternalInput")
    if bench_mode:
        out = nc.dram_tensor("out_scratch", [BPC, COUT, HW], BF16)
        token = nc.dram_tensor("token", [1, 1], FP32, kind="ExternalOutput")
    else:
        out = nc.dram_tensor("out", [BPC, COUT, HW], BF16, kind="ExternalOutput")
        token = None

    # The two HWDGE rings (SP + ACT) stream x in parallel.
    x_dma_engines = [nc.sync, nc.scalar]

    with TileContext(nc) as tc:
        with (
            tc.tile_pool(name="consts", bufs=1) as cpool,
            tc.tile_pool(name="xs", bufs=x_bufs) as xpool,
            tc.tile_pool(name="os", bufs=o_bufs) as opool,
            tc.tile_pool(name="ps", bufs=psum_bufs, space="PSUM") as pspool,
        ):
            wT_sb = cpool.tile([P, KT, COUT], FP32)
            nc.sync.dma_start(out=wT_sb[:], in_=wT[:].rearrange("(t p) o -> p t o", p=P))
            sT_sb = cpool.tile([P, KT, BPC], FP32)
            nc.scalar.dma_start(
                out=sT_sb[:], in_=styleT[:].rearrange("(t p) b -> p t b", p=P)
            )
            # Per-sample modulated (transposed) weights: mw[p, b, t, o], bf16.
            mw_sb = cpool.tile([P, BPC, KT, COUT], BF16)
            for b in range(BPC):
                for t in range(KT):
                    nc.vector.tensor_scalar_mul(
                        mw_sb[:, b, t, :], wT_sb[:, t, :], sT_sb[:, t, b : b + 1]
                    )

            dma_i = 0
            for _rep in range(reps):
                for b in range(BPC):
                    xq = []
                    for q in range(NQ):
                        xt = xpool.tile([P, KT * NC], BF16, tag="xt")
                        eng = x_dma_engines[dma_i % len(x_dma_engines)]
                        dma_i += 1
                        eng.dma_start(out=xt[:], in_=x[b * NQ + q])
                        xq.append(xt)

                    ot = opool.tile([P, HW], BF16, tag="ot")
                    for q in range(NQ):
                        ps = pspool.tile([P, NC], FP32, tag="ps")
                        for t in range(KT):
                            nc.tensor.matmul(
                                ps[:],
                                mw_sb[:, b, t, :],
                                xq[q][:, t * NC : (t + 1) * NC],
                                start=(t == 0),
                                stop=(t == KT - 1),
                            )
                        nc.vector.tensor_copy(
                            out=ot[:, q * NC : (q + 1) * NC], in_=ps[:]
                        )
                        if (q + 1) % out_every == 0:
                            lo = (q + 1 - out_every) * NC
                            hi = (q + 1) * NC
                            nc.gpsimd.dma_start(out=out[b, :, lo:hi], in_=ot[:, lo:hi])
            if token is not None:
                nc.gpsimd.dma_start(out=token[:], in_=wT_sb[:1, 0, :1])

    _split_sync_waits(nc)
    return nc


_NC_CACHE: bass.Bass | None = None


def _get_nc() -> bass.Bass:
    global _NC_CACHE
    if _NC_CACHE is None:
        _NC_CACHE = build_kernel()
    return _NC_CACHE


def make_in_maps(x: np.ndarray, style: np.ndarray, weight: np.ndarray):
    # [B, CIN, HW] -> [B, NQ, P, KT, NC] (channel c = t*P + p, col = q*NC + n)
    xb = np.asarray(x, dtype=np.float32).astype(NP_BF16)
    xr = np.ascontiguousarray(
        xb.reshape(B, KT, P, NQ, NC).transpose(0, 3, 2, 1, 4)
    ).reshape(B, NQ, P, KT * NC)
    styleT = np.ascontiguousarray(np.asarray(style, dtype=np.float32).T)  # [CIN, B]
    wT = np.ascontiguousarray(np.asarray(weight, dtype=np.float32).T)  # [CIN, COUT]
    in_maps = []
    for c in range(N_CORES):
        sl = slice(c * BPC, (c + 1) * BPC)
        in_maps.append(
            {
                "x": xr[sl].reshape(BPC * NQ, P, KT * NC),
                "styleT": np.ascontiguousarray(styleT[:, sl]),
                "wT": wT,
            }
        )
    return in_maps


def gather_out(results) -> np.ndarray:
    out = np.empty((B, COUT, H, W), dtype=np.float32)
    for c in range(N_CORES):
        out[c * BPC : (c + 1) * BPC] = (
            results[c]["out"].astype(np.float32).reshape(BPC, COUT, H, W)
        )
    return out


def kernel(x: np.ndarray, style: np.ndarray, weight: np.ndarray) -> np.ndarray:
    nc = _get_nc()
    in_maps = make_in_maps(x, style, weight)
    res = run_bass_kernel_spmd(nc, in_maps, core_ids=list(range(N_CORES)))
    return gather_out(res.results)


# revision 20
# speedup vs baseline: 2.3741x; 1.2699x over previous
"""Modulated 1x1 conv (ModConv) on 8 Trainium2 NeuronCores.

out[b,o,h,w] = sum_c (style[b,c] * weight[o,c]) * x[b,c,h,w]

Strategy: data parallel over the batch — 2 samples per core. The
problem is memory-bound, so both x and the output travel as bf16
(correctness gate is 2e-2; bf16 GEMM error is ~3e-3), halving traffic
vs fp32: 8.4 MB in + 2.1 MB out per core. Measured input streaming on
the two HWDGE rings is ~565 GB/s aggregate -> ~19 us roofline for
10.5 MB; PE time (bf16, 2 samples x 4 k-tiles x 4096 cols = 32768
cycles @ 2.4 GHz ~ 13.7 us) hides under the DMA.

Layouts are chosen so every DMA is fully contiguous in DRAM on both
ends (the host pre-permutes x to [b, q, p, t, n] chunks and
un-permutes the chunk-major output afterwards; p = channel % 128,
t = channel // 128). x chunks alternate between the SP and ACT HWDGE
rings. Per 512-col n-tile: 4 PSUM-accumulated bf16 matmuls, then DVE
copies the fp32 PSUM bank out as bf16. Output DMAs also ride the two
HWDGE rings, but their issue is DEFERRED by `out_defer` samples so the
ring sequencer (which prefetches x up to two samples ahead) never
stalls on the not-yet-ready DVE copy semaphore — non-deferred variants
measure bimodal 22-34 us; deferred ones are stable. The gpsimd SWDGE
path is avoided entirely: its software descriptor generation (~128
descriptors per out chunk) costs ~4 us per DMA.
"""

import numpy as np
import ml_dtypes

import concourse.bass as bass
import concourse.mybir as mybir
from concourse.bass_utils import run_bass_kernel_spmd
from concourse.tile import TileContext

B, CIN, COUT, H, W = 16, 512, 128, 64, 64
HW = H * W
N_CORES = 8
BPC = B // N_CORES  # samples per core
P = 128
KT = CIN // P  # k-tiles per contraction
NQ = 8  # HW-chunks per sample
NC = HW // NQ  # columns per chunk (= one PSUM bank of fp32)
FP32 = mybir.dt.float32
BF16 = mybir.dt.bfloat16
NP_BF16 = ml_dtypes.bfloat16

# This container's walrus (public-SDK build) accepts at most one sync
# wait command per instruction; Tile's sem assignment attaches one wait
# per depended-on proc. Hoist the excess onto dedicated wait
# instructions (the same InstEventSemaphore a bass `wait_ge` emits)
# immediately before the over-subscribed instruction on its own engine.
MAX_WAITS_PER_INST = 1


def _split_sync_waits(nc: bass.Bass, limit: int = MAX_WAITS_PER_INST) -> int:
    n_split = 0
    for f in nc.m.functions:
        for bb in f.blocks:
            out = []
            for ins in bb.instructions:
                si = getattr(ins, "sync_info", None)
                if si is not None and si.on_wait and len(si.on_wait) > limit:
                    waits = list(si.on_wait)
                    for w in waits[:-limit]:
                        n_split += 1
                        es = mybir.InstEventSemaphore(
                            name=f"{ins.name}-ws{n_split}",
                            opcode="EventSemaphore",
                            engine=ins.engine,
                            sync_info=mybir.SyncInfo(on_wait=[w], on_update=[]),
                        )
                        nc.register_instruction(es, overwrite=True)
                        out.append(es)
                    si.on_wait = waits[-limit:]
                out.append(ins)
            bb.instructions[:] = out
    return n_split


def build_kernel(
    reps: int = 1,
    bench_mode: bool = False,
    nq: int = NQ,  # x DMA chunks per sample (8 -> 512 KB each)
    x_bufs: int | None = None,  # default: 2 samples in flight + 1 slot
    psum_bufs: int = 4,
    o_bufs: int = 2,
    out_every: int = 4,  # 512-col n-tiles per output DMA (4 -> 512 KB bf16)
    skip_out: bool = False,
    skip_compute: bool = False,
    out_hwdge: bool = False,
    out_contig: bool = False,  # chunk-contiguous DRAM layout for out
    out_defer: int = 0,  # issue sample b's out DMAs during sample b+defer's x phase
    x_engines: int = 2,
) -> bass.Bass:
    """reps>1 replicates the whole per-sample pipeline in-program (same
    inputs, outputs rewritten) — used only by the bench to measure
    steady-state per-iteration time with per-call overhead cancelled.
    bench_mode writes the big output to internal DRAM and exposes only a
    4-byte token output, so per-call tunnel traffic is negligible."""
    ncc = HW // nq  # columns per x chunk
    ntiles_per_chunk = ncc // NC
    if x_bufs is None:
        x_bufs = 2 * nq + 1
    nc = bass.Bass()
    # x chunk ci = b*nq + q: [P, KT*ncc] fully contiguous in DRAM.
    x = nc.dram_tensor("x", [BPC * nq, P, KT * ncc], BF16, kind="ExternalInput")
    styleT = nc.dram_tensor("styleT", [CIN, BPC], FP32, kind="ExternalInput")
    wT = nc.dram_tensor("wT", [CIN, COUT], FP32, kind="ExternalInput")
    n_ochunks = HW // (out_every * NC)
    oshape = (
        [BPC * n_ochunks, COUT, out_every * NC] if out_contig else [BPC, COUT, HW]
    )
    if bench_mode:
        out = nc.dram_tensor("out_scratch", oshape, BF16)
        token = nc.dram_tensor("token", [1, 1], FP32, kind="ExternalOutput")
    else:
        out = nc.dram_tensor("out", oshape, BF16, kind="ExternalOutput")
        token = None

    # The two HWDGE rings (SP + ACT) stream x in parallel.
    x_dma_engines = [nc.sync, nc.scalar][:x_engines]

    with TileContext(nc) as tc:
        with (
            tc.tile_pool(name="consts", bufs=1) as cpool,
            tc.tile_pool(name="xs", bufs=x_bufs) as xpool,
            tc.tile_pool(name="os", bufs=o_bufs) as opool,
            tc.tile_pool(name="ps", bufs=psum_bufs, space="PSUM") as pspool,
        ):
            wT_sb = cpool.tile([P, KT, COUT], FP32)
            nc.sync.dma_start(out=wT_sb[:], in_=wT[:].rearrange("(t p) o -> p t o", p=P))
            sT_sb = cpool.tile([P, KT, BPC], FP32)
            nc.scalar.dma_start(
                out=sT_sb[:], in_=styleT[:].rearrange("(t p) b -> p t b", p=P)
            )
            # Per-sample modulated (transposed) weights: mw[p, b, t, o], bf16.
            mw_sb = cpool.tile([P, BPC, KT, COUT], BF16)
            for b in range(BPC):
                for t in range(KT):
                    nc.vector.tensor_scalar_mul(
                        mw_sb[:, b, t, :], wT_sb[:, t, :], sT_sb[:, t, b : b + 1]
                    )

            dma_i = 0
            pending: list[list] = []  # deferred (dst, src) out DMAs per sample

            def flush_outs(outs):
                nonlocal dma_i
                for dst, src in outs:
                    if out_hwdge:
                        oeng = x_dma_engines[dma_i % len(x_dma_engines)]
                        dma_i += 1
                    else:
                        oeng = nc.gpsimd
                    oeng.dma_start(out=dst, in_=src)

            for _rep in range(reps):
                for b in range(BPC):
                    xq = []
                    for q in range(nq):
                        xt = xpool.tile([P, KT * ncc], BF16, tag="xt")
                        eng = x_dma_engines[dma_i % len(x_dma_engines)]
                        dma_i += 1
                        eng.dma_start(out=xt[:], in_=x[b * nq + q])
                        xq.append(xt)
                    if out_defer and len(pending) >= out_defer:
                        flush_outs(pending.pop(0))
                    if skip_compute:
                        continue

                    ot = opool.tile([P, HW], BF16, tag="ot")
                    my_outs = []
                    for n in range(HW // NC):
                        q, j = divmod(n, ntiles_per_chunk)
                        ps = pspool.tile([P, NC], FP32, tag="ps")
                        for t in range(KT):
                            nc.tensor.matmul(
                                ps[:],
                                mw_sb[:, b, t, :],
                                xq[q][:, t * ncc + j * NC : t * ncc + (j + 1) * NC],
                                start=(t == 0),
                                stop=(t == KT - 1),
                            )
                        nc.vector.tensor_copy(
                            out=ot[:, n * NC : (n + 1) * NC], in_=ps[:]
                        )
                        if not skip_out and (n + 1) % out_every == 0:
                            lo = (n + 1 - out_every) * NC
                            hi = (n + 1) * NC
                            if out_contig:
                                dst = out[b * n_ochunks + (n + 1) // out_every - 1]
                            else:
                                dst = out[b, :, lo:hi]
                            if out_defer:
                                my_outs.append((dst, ot[:, lo:hi]))
                            else:
                                flush_outs([(dst, ot[:, lo:hi])])
                    if my_outs:
                        pending.append(my_outs)
            for outs in pending:
                flush_outs(outs)
            if token is not None:
                nc.gpsimd.dma_start(out=token[:], in_=wT_sb[:1, 0, :1])

    _split_sync_waits(nc)
    return nc


_NC_CACHE: bass.Bass | None = None

# The production configuration (used by kernel()/gather_out/unscramble_out).
DEFAULT_CFG: dict = dict(
    nq=4,
    x_bufs=13,
    out_contig=True,
    out_hwdge=True,
    out_defer=2,
    o_bufs=4,
)


def _get_nc() -> bass.Bass:
    global _NC_CACHE
    if _NC_CACHE is None:
        _NC_CACHE = build_kernel(**DEFAULT_CFG)
    return _NC_CACHE


def unscramble_out(arr: np.ndarray) -> np.ndarray:
    """Per-core raw out tensor -> [BPC, COUT, HW] float32."""
    a = np.asarray(arr).astype(np.float32)
    if DEFAULT_CFG.get("out_contig", False):
        oc = DEFAULT_CFG.get("out_every", 4) * NC
        a = a.reshape(BPC, HW // oc, COUT, oc).transpose(0, 2, 1, 3)
    return a.reshape(BPC, COUT, HW)


def make_in_maps(
    x: np.ndarray, style: np.ndarray, weight: np.ndarray, nq: int | None = None
):
    # [B, CIN, HW] -> [B, nq, P, KT, ncc] (channel c = t*P + p, col = q*ncc + n)
    if nq is None:
        nq = DEFAULT_CFG.get("nq", NQ)
    ncc = HW // nq
    xb = np.asarray(x, dtype=np.float32).astype(NP_BF16)
    xr = np.ascontiguousarray(
        xb.reshape(B, KT, P, nq, ncc).transpose(0, 3, 2, 1, 4)
    ).reshape(B, nq, P, KT * ncc)
    styleT = np.ascontiguousarray(np.asarray(style, dtype=np.float32).T)  # [CIN, B]
    wT = np.ascontiguousarray(np.asarray(weight, dtype=np.float32).T)  # [CIN, COUT]
    in_maps = []
    for c in range(N_CORES):
        sl = slice(c * BPC, (c + 1) * BPC)
        in_maps.append(
            {
                "x": xr[sl].reshape(BPC * nq, P, KT * ncc),
                "styleT": np.ascontiguousarray(styleT[:, sl]),
                "wT": wT,
            }
        )
    return in_maps


def gather_out(results) -> np.ndarray:
    out = np.empty((B, COUT, H, W), dtype=np.float32)
    for c in range(N_CORES):
        out[c * BPC : (c + 1) * BPC] = unscramble_out(results[c]["out"]).reshape(
            BPC, COUT, H, W
        )
    return out


def kernel(x: np.ndarray, style: np.ndarray, weight: np.ndarray) -> np.ndarray:
    nc = _get_nc()
    in_maps = make_in_maps(x, style, weight)
    res = run_bass_kernel_spmd(nc, in_maps, core_ids=list(range(N_CORES)))
    return gather_out(res.results)


# revision 25
# speedup vs baseline: 2.4463x; 1.0304x over previous
"""Modulated 1x1 conv (ModConv) on 8 Trainium2 NeuronCores.

out[b,o,h,w] = sum_c (style[b,c] * weight[o,c]) * x[b,c,h,w]

Strategy: data parallel over the batch — 2 samples per core. The
problem is memory-bound, so both x and the output travel as bf16
(correctness gate is 2e-2; bf16 GEMM error is ~3e-3), halving traffic
vs fp32: 8.4 MB in + 2.1 MB out per core. Measured input streaming on
the two HWDGE rings is ~565 GB/s aggregate -> ~19 us roofline for
10.5 MB; PE time (bf16, 2 samples x 4 k-tiles x 4096 cols = 32768
cycles @ 2.4 GHz ~ 13.7 us) hides under the DMA.

Layouts are chosen so every DMA is fully contiguous in DRAM on both
ends (the host pre-permutes x to [b, q, p, t, n] chunks and
un-permutes the chunk-major output afterwards; p = channel % 128,
t = channel // 128). x chunks alternate between the SP and ACT HWDGE
rings. Per 512-col n-tile: 4 PSUM-accumulated bf16 matmuls, then DVE
copies the fp32 PSUM bank out as bf16. Output DMAs also ride the two
HWDGE rings, but their issue is DEFERRED by `out_defer` samples so the
ring sequencer (which prefetches x up to two samples ahead) never
stalls on the not-yet-ready DVE copy semaphore — non-deferred variants
measure bimodal 22-34 us; deferred ones are stable. The gpsimd SWDGE
path is avoided entirely: its software descriptor generation (~128
descriptors per out chunk) costs ~4 us per DMA.
"""

import numpy as np
import ml_dtypes

import concourse.bass as bass
import concourse.mybir as mybir
from concourse.bass_utils import run_bass_kernel_spmd
from concourse.tile import TileContext

B, CIN, COUT, H, W = 16, 512, 128, 64, 64
HW = H * W
N_CORES = 8
BPC = B // N_CORES  # samples per core
P = 128
KT = CIN // P  # k-tiles per contraction
NQ = 8  # HW-chunks per sample
NC = HW // NQ  # columns per chunk (= one PSUM bank of fp32)
FP32 = mybir.dt.float32
BF16 = mybir.dt.bfloat16
NP_BF16 = ml_dtypes.bfloat16

# This container's walrus (public-SDK build) accepts at most one sync
# wait command per instruction; Tile's sem assignment attaches one wait
# per depended-on proc. Hoist the excess onto dedicated wait
# instructions (the same InstEventSemaphore a bass `wait_ge` emits)
# immediately before the over-subscribed instruction on its own engine.
MAX_WAITS_PER_INST = 1


def _split_sync_waits(nc: bass.Bass, limit: int = MAX_WAITS_PER_INST) -> int:
    n_split = 0
    for f in nc.m.functions:
        for bb in f.blocks:
            out = []
            for ins in bb.instructions:
                si = getattr(ins, "sync_info", None)
                if si is not None and si.on_wait and len(si.on_wait) > limit:
                    waits = list(si.on_wait)
                    for w in waits[:-limit]:
                        n_split += 1
                        es = mybir.InstEventSemaphore(
                            name=f"{ins.name}-ws{n_split}",
                            opcode="EventSemaphore",
                            engine=ins.engine,
                            sync_info=mybir.SyncInfo(on_wait=[w], on_update=[]),
                        )
                        nc.register_instruction(es, overwrite=True)
                        out.append(es)
                    si.on_wait = waits[-limit:]
                out.append(ins)
            bb.instructions[:] = out
    return n_split


def build_kernel(
    reps: int = 1,
    bench_mode: bool = False,
    nq: int = NQ,  # x DMA chunks per sample (8 -> 512 KB each)
    x_bufs: int | None = None,  # default: 2 samples in flight + 1 slot
    psum_bufs: int = 4,
    o_bufs: int = 2,
    out_every: int = 4,  # 512-col n-tiles per output DMA (4 -> 512 KB bf16)
    skip_out: bool = False,
    skip_compute: bool = False,
    out_hwdge: bool = False,
    out_contig: bool = False,  # chunk-contiguous DRAM layout for out
    out_defer: int = 0,  # issue sample b's out DMAs during sample b+defer's x phase
    x_engines: int = 2,
    mm_n: int = NC,  # matmul free dim (1024 = 2-bank PSUM tiles, half the MMs)
) -> bass.Bass:
    """reps>1 replicates the whole per-sample pipeline in-program (same
    inputs, outputs rewritten) — used only by the bench to measure
    steady-state per-iteration time with per-call overhead cancelled.
    bench_mode writes the big output to internal DRAM and exposes only a
    4-byte token output, so per-call tunnel traffic is negligible."""
    ncc = HW // nq  # columns per x chunk
    ntiles_per_chunk = ncc // mm_n
    if x_bufs is None:
        x_bufs = 2 * nq + 1
    nc = bass.Bass()
    # x chunk ci = b*nq + q: [P, KT*ncc] fully contiguous in DRAM.
    x = nc.dram_tensor("x", [BPC * nq, P, KT * ncc], BF16, kind="ExternalInput")
    styleT = nc.dram_tensor("styleT", [CIN, BPC], FP32, kind="ExternalInput")
    wT = nc.dram_tensor("wT", [CIN, COUT], FP32, kind="ExternalInput")
    n_ochunks = HW // (out_every * mm_n)
    oshape = (
        [BPC * n_ochunks, COUT, out_every * mm_n] if out_contig else [BPC, COUT, HW]
    )
    if bench_mode:
        out = nc.dram_tensor("out_scratch", oshape, BF16)
        token = nc.dram_tensor("token", [1, 1], FP32, kind="ExternalOutput")
    else:
        out = nc.dram_tensor("out", oshape, BF16, kind="ExternalOutput")
        token = None

    # The two HWDGE rings (SP + ACT) stream x in parallel.
    x_dma_engines = [nc.sync, nc.scalar][:x_engines]

    with TileContext(nc) as tc:
        with (
            tc.tile_pool(name="consts", bufs=1) as cpool,
            tc.tile_pool(name="xs", bufs=x_bufs) as xpool,
            tc.tile_pool(name="os", bufs=o_bufs) as opool,
            tc.tile_pool(name="ps", bufs=psum_bufs, space="PSUM") as pspool,
        ):
            wT_sb = cpool.tile([P, KT, COUT], FP32)
            nc.sync.dma_start(out=wT_sb[:], in_=wT[:].rearrange("(t p) o -> p t o", p=P))
            sT_sb = cpool.tile([P, KT, BPC], FP32)
            nc.scalar.dma_start(
                out=sT_sb[:], in_=styleT[:].rearrange("(t p) b -> p t b", p=P)
            )
            # Per-sample modulated (transposed) weights: mw[p, b, t, o], bf16.
            mw_sb = cpool.tile([P, BPC, KT, COUT], BF16)
            for b in range(BPC):
                for t in range(KT):
                    nc.vector.tensor_scalar_mul(
                        mw_sb[:, b, t, :], wT_sb[:, t, :], sT_sb[:, t, b : b + 1]
                    )

            dma_i = 0
            pending: list[list] = []  # deferred (dst, src) out DMAs per sample

            def flush_outs(outs):
                nonlocal dma_i
                for dst, src in outs:
                    if out_hwdge:
                        oeng = x_dma_engines[dma_i % len(x_dma_engines)]
                        dma_i += 1
                    else:
                        oeng = nc.gpsimd
                    oeng.dma_start(out=dst, in_=src)

            for _rep in range(reps):
                for b in range(BPC):
                    xq = []
                    for q in range(nq):
                        xt = xpool.tile([P, KT * ncc], BF16, tag="xt")
                        eng = x_dma_engines[dma_i % len(x_dma_engines)]
                        dma_i += 1
                        eng.dma_start(out=xt[:], in_=x[b * nq + q])
                        xq.append(xt)
                    if out_defer and len(pending) >= out_defer:
                        flush_outs(pending.pop(0))
                    if skip_compute:
                        continue

                    ot = opool.tile([P, HW], BF16, tag="ot")
                    my_outs = []
                    for n in range(HW // mm_n):
                        q, j = divmod(n, ntiles_per_chunk)
                        ps = pspool.tile([P, mm_n], FP32, tag="ps")
                        for t in range(KT):
                            nc.tensor.matmul(
                                ps[:],
                                mw_sb[:, b, t, :],
                                xq[q][
                                    :, t * ncc + j * mm_n : t * ncc + (j + 1) * mm_n
                                ],
                                start=(t == 0),
                                stop=(t == KT - 1),
                            )
                        nc.vector.tensor_copy(
                            out=ot[:, n * mm_n : (n + 1) * mm_n], in_=ps[:]
                        )
                        if not skip_out and (n + 1) % out_every == 0:
                            lo = (n + 1 - out_every) * mm_n
                            hi = (n + 1) * mm_n
                            if out_contig:
                                dst = out[b * n_ochunks + (n + 1) // out_every - 1]
                            else:
                                dst = out[b, :, lo:hi]
                            if out_defer:
                                my_outs.append((dst, ot[:, lo:hi]))
                            else:
                                flush_outs([(dst, ot[:, lo:hi])])
                    if my_outs:
                        pending.append(my_outs)
            for outs in pending:
                flush_outs(outs)
            if token is not None:
                nc.gpsimd.dma_start(out=token[:], in_=wT_sb[:1, 0, :1])

    _split_sync_waits(nc)
    return nc


_NC_CACHE: bass.Bass | None = None

# The production configuration (used by kernel()/gather_out/unscramble_out).
DEFAULT_CFG: dict = dict(
    nq=4,
    x_bufs=13,
    out_contig=True,
    out_hwdge=True,
    out_defer=2,
    o_bufs=4,
)


def _get_nc() -> bass.Bass:
    global _NC_CACHE
    if _NC_CACHE is None:
        _NC_CACHE = build_kernel(**DEFAULT_CFG)
    return _NC_CACHE


def unscramble_out(arr: np.ndarray) -> np.ndarray:
    """Per-core raw out tensor -> [BPC, COUT, HW] float32."""
    a = np.asarray(arr).astype(np.float32)
    if DEFAULT_CFG.get("out_contig", False):
        oc = DEFAULT_CFG.get("out_every", 4) * DEFAULT_CFG.get("mm_n", NC)
        a = a.reshape(BPC, HW // oc, COUT, oc).transpose(0, 2, 1, 3)
    return a.reshape(BPC, COUT, HW)


def make_in_maps(
    x: np.ndarray, style: np.ndarray, weight: np.ndarray, nq: int | None = None
):
    # [B, CIN, HW] -> [B, nq, P, KT, ncc] (channel c = t*P + p, col = q*ncc + n)
    if nq is None:
        nq = DEFAULT_CFG.get("nq", NQ)
    ncc = HW // nq
    xb = np.asarray(x, dtype=np.float32).astype(NP_BF16)
    xr = np.ascontiguousarray(
        xb.reshape(B, KT, P, nq, ncc).transpose(0, 3, 2, 1, 4)
    ).reshape(B, nq, P, KT * ncc)
    styleT = np.ascontiguousarray(np.asarray(style, dtype=np.float32).T)  # [CIN, B]
    wT = np.ascontiguousarray(np.asarray(weight, dtype=np.float32).T)  # [CIN, COUT]
    in_maps = []
    for c in range(N_CORES):
        sl = slice(c * BPC, (c + 1) * BPC)
        in_maps.append(
            {
                "x": xr[sl].reshape(BPC * nq, P, KT * ncc),
                "styleT": np.ascontiguousarray(styleT[:, sl]),
                "wT": wT,
            }
        )
    return in_maps


def gather_out(results) -> np.ndarray:
    out = np.empty((B, COUT, H, W), dtype=np.float32)
    for c in range(N_CORES):
        out[c * BPC : (c + 1) * BPC] = unscramble_out(results[c]["out"]).reshape(
            BPC, COUT, H, W
        )
    return out


def kernel(x: np.ndarray, style: np.ndarray, weight: np.ndarray) -> np.ndarray:
    nc = _get_nc()
    in_maps = make_in_maps(x, style, weight)
    res = run_bass_kernel_spmd(nc, in_maps, core_ids=list(range(N_CORES)))
    return gather_out(res.results)
